# revision 25
# baseline (speedup 1.0000x reference)
"""Trainium2 Bass kernel for nn_BoneRefusion (17-group BoneMLP over [B,T,16,3]).

v7 design (pure data parallel over batch, 8 cores):
  - Host packs per-core inputs feature-major with 2-set packing:
      x2 [98, S] bf16, S = tokens_per_core/2. Rows 0-47 = set A features,
      row 48 = ones (bakes b1 into layer 1), rows 49-96 = set B,
      row 97 = ones. Column j holds tokens (j, S+j).
  - On chip x lives as [128, N]: set A at partitions 0-48, set B at
    64-112, so the PE runs in 64x32 array-tiling mode: row group 0 = A,
    row group 1 = B, 8 concurrent 64x32 tiles. Blocks are N=486
    pair-columns, 64 blocks per core.
  - A prologue of back-to-back dummy matmuls warms the PE HAM clock
    gate once (4/8 -> 8/8); steady-state gaps are too short to
    re-throttle, so every real matmul runs at 2.4 GHz.
  - PSUM (f32), all 8 banks:
      pH [128,4,512]: A h0-127 | B h0-127 | A h128-255 | B h128-255
      pV [128,2,512]: group-16 tails of 4 blocks (A | B), rows 32*(p%4)
      pO [128,2,512]: layer-2 outs row-group-0 | row-group-1
  - Evacuation (TRN2 floor: fp32 PSUM = 1 elem/cycle/lane): DVE handles
    the A banks -> hA, ACT the B banks -> hB (separate destination
    tiles per engine avoid cross-engine false dependencies); outs split
    ob8a (ACT) / ob8b (DVE); h and outs leave as bf16.
  - Layer 2 is software-pipelined one block behind layer 1.
  - The group-16 16x3 epilogue runs on the host from the exported tail
    h (o3, gpsimd SWDGE); b2 is added on the host during unshard.
  - Input loads and batched main-out stores ride the sync DGE queue.
"""

import sys

import numpy as np
import ml_dtypes

sys.path.insert(0, "/opt/trn_rl_repo")

import concourse.bass as bass
import concourse.mybir as mybir
import concourse.tile as tile
from concourse import bacc
from concourse.bass_utils import run_bass_kernel_spmd

BF16 = mybir.dt.bfloat16
F32 = mybir.dt.float32
BF16_NP = ml_dtypes.bfloat16

NG = 17          # groups
HID = 16         # hidden per group
B, T, NJ, C = 2048, 243, 16, 3
NF = NJ * C      # 48 input features per token
NCORES = 8
BC = B // NCORES           # batches per core
TC = BC * T                # tokens per core
S = TC // 2                # token pairs per core (2-set packing)
KX = 2 * (NF + 1)          # 98 input rows in DRAM
NBLK = 486                 # pair-columns per block
NBLOCKS = S // NBLK        # 64
LBATCH = 8                 # blocks per input-load / output-store batch
S4 = S // 4                # o3 columns (tail batches of 4 blocks)


def _host_weights(W1, b1, W2, b2, idx):
    """Stationaries: w1l [128, 9*32] bf16, w2l [128, 48] bf16."""
    W1 = np.asarray(W1, np.float32)
    b1 = np.asarray(b1, np.float32)
    W2 = np.asarray(W2, np.float32)
    idx = np.asarray(idx)

    # Scatter per-group [12, 16] W1 blocks into the 48-feature space.
    # Padded limb rows of W1 are already zero, so += handles duplicates.
    w1full = np.zeros((NF, NG * HID), np.float32)
    for g in range(NG):
        for j in range(4):
            r = int(idx[g, j]) * C
            w1full[r:r + C, g * HID:(g + 1) * HID] += W1[g, j * C:(j + 1) * C, :]
    b1flat = b1.reshape(NG * HID)

    # Layer-1 chunk stationaries [49, 32] x 9 (chunk 8 = group-16 tail,
    # cols 16-31 stay zero), duplicated to rows 64-112 for the
    # row-group-1 (set B) array tiles.
    w1l = np.zeros((128, 9 * 32), np.float32)
    for c in range(8):
        w1l[0:NF, 32 * c:32 * c + 32] = w1full[:, 32 * c:32 * c + 32]
        w1l[NF, 32 * c:32 * c + 32] = b1flat[32 * c:32 * c + 32]
    w1l[0:NF, 8 * 32:8 * 32 + 16] = w1full[:, 256:272]
    w1l[NF, 8 * 32:8 * 32 + 16] = b1flat[256:272]
    w1l[64:128, :] = w1l[0:64, :]

    # Layer-2 stationaries: M1-M4 [64, 12] at cols 12k (groups 4k..4k+3),
    # duplicated to rows 64-127.
    w2l = np.zeros((128, 48), np.float32)
    for k in range(4):
        for j in range(4):
            g = 4 * k + j
            w2l[16 * j:16 * j + 16, 12 * k + 3 * j:12 * k + 3 * j + 3] = W2[g]
    w2l[64:128, :] = w2l[0:64, :]

    return w1l.astype(BF16_NP), w2l.astype(BF16_NP)


def _build_nc(repeat=1):
    nc = bacc.Bacc(
        "TRN2", target_bir_lowering=False, debug=False, num_devices=NCORES,
    )
    x2 = nc.dram_tensor("x2", [KX, S], BF16, kind="ExternalInput").ap()
    w1 = nc.dram_tensor("w1", [128, 9 * 32], BF16, kind="ExternalInput").ap()
    w2 = nc.dram_tensor("w2", [128, 48], BF16, kind="ExternalInput").ap()
    # Outputs bf16. o12[g, r, q]: layer-2 outs from psum col group g
    # (partitions 32g..32g+12), q = psum bank (0 = row-group-0 matmuls,
    # 1 = row-group-1). o3[32j+k, a, 486t:486t+486] = relu'd group-16
    # hidden row k (k<16) of set a for block 4t+j.
    o12 = nc.dram_tensor("o12", [4, 12, 2, S], BF16, kind="ExternalOutput").ap()
    o3 = nc.dram_tensor("o3", [128, 2, S4], BF16, kind="ExternalOutput").ap()

    with tile.TileContext(nc) as tc:
        with (
            tc.tile_pool(name="singles", bufs=1) as singles,
            tc.tile_pool(name="xin", bufs=2) as xin,
            tc.tile_pool(name="ha", bufs=2) as hap,
            tc.tile_pool(name="hb", bufs=2) as hbp,
            tc.tile_pool(name="tsb", bufs=2) as tsb,
            tc.tile_pool(name="osa", bufs=2) as osa,
            tc.tile_pool(name="osb", bufs=2) as osb,
            tc.tile_pool(name="hps", bufs=1, space="PSUM") as hps,
            tc.tile_pool(name="vps", bufs=1, space="PSUM") as vps,
        ):
            w1_sb = singles.tile([128, 9, 32], BF16)
            nc.sync.dma_start(w1_sb, w1.rearrange("k (c m) -> k c m", c=9))
            w2_sb = singles.tile([128, 48], BF16)
            nc.sync.dma_start(w2_sb, w2)

            WARMUP = False
            if WARMUP:
                # HAM warm-up: back-to-back matmuls on array tile (0,0)
                # into rotating pH banks (no deps -> pure PE-FIFO stream).
                pH0 = hps.tile([128, 4, 512], F32, tag="pH")
                wrm = w1_sb[0:49].rearrange("p c m -> p (c m)")
                for k in range(10):
                    nc.tensor.matmul(
                        pH0[0:32, k % 4, 0:288], lhsT=w1_sb[0:49, 0],
                        rhs=wrm[:, 0:288], start=True, stop=True,
                        tile_position=(0, 0))

            def layer2(p, hA, hB, ob8a, ob8b):
                pO = hps.tile([128, 2, 512], F32, tag="pO")
                nc.tensor.matmul(pO[0:12, 0, 0:NBLK], lhsT=w2_sb[0:64, 0:12],
                                 rhs=hA[0:64, 0], start=True, stop=True,
                                 tile_position=(0, 0))
                nc.tensor.matmul(pO[0:12, 1, 0:NBLK], lhsT=w2_sb[64:128, 12:24],
                                 rhs=hA[64:128, 0], start=True, stop=True,
                                 tile_position=(64, 0))
                nc.tensor.matmul(pO[32:44, 0, 0:NBLK], lhsT=w2_sb[0:64, 24:36],
                                 rhs=hA[0:64, 1], start=True, stop=True,
                                 tile_position=(0, 32))
                nc.tensor.matmul(pO[32:44, 1, 0:NBLK], lhsT=w2_sb[64:128, 36:48],
                                 rhs=hA[64:128, 1], start=True, stop=True,
                                 tile_position=(64, 32))
                nc.tensor.matmul(pO[64:76, 0, 0:NBLK], lhsT=w2_sb[0:64, 0:12],
                                 rhs=hB[0:64, 0], start=True, stop=True,
                                 tile_position=(0, 64))
                nc.tensor.matmul(pO[64:76, 1, 0:NBLK], lhsT=w2_sb[64:128, 12:24],
                                 rhs=hB[64:128, 0], start=True, stop=True,
                                 tile_position=(64, 64))
                nc.tensor.matmul(pO[96:108, 0, 0:NBLK], lhsT=w2_sb[0:64, 24:36],
                                 rhs=hB[0:64, 1], start=True, stop=True,
                                 tile_position=(0, 96))
                nc.tensor.matmul(pO[96:108, 1, 0:NBLK], lhsT=w2_sb[64:128, 36:48],
                                 rhs=hB[64:128, 1], start=True, stop=True,
                                 tile_position=(64, 96))
                # out evac split across engines into per-engine tiles
                nc.scalar.activation(
                    out=ob8a[:, p % LBATCH, :], in_=pO[:, 0, 0:NBLK],
                    func=mybir.ActivationFunctionType.Identity)
                nc.vector.tensor_scalar(
                    ob8b[:, p % LBATCH, :], pO[:, 1, 0:NBLK], 0.0, None,
                    mybir.AluOpType.add)

            prev = None          # (p, hA, hB) awaiting layer 2
            ob8a = ob8b = None
            pV = None
            for _rep in range(repeat):
              for p in range(NBLOCKS):
                if p % LBATCH == 0:
                    xt8 = xin.tile([128, LBATCH * NBLK], BF16, tag="xt")
                    lo = p * NBLK
                    nc.sync.dma_start(xt8[0:49, :],
                                      x2[0:49, lo:lo + LBATCH * NBLK])
                    nc.sync.dma_start(xt8[64:113, :],
                                      x2[49:98, lo:lo + LBATCH * NBLK])
                if p % 4 == 0:
                    pV = vps.tile([128, 2, 512], F32, tag="pV")
                half = p % LBATCH
                xA = xt8[0:49, half * NBLK:(half + 1) * NBLK]
                xB = xt8[64:113, half * NBLK:(half + 1) * NBLK]
                pH = hps.tile([128, 4, 512], F32, tag="pH")
                hA = hap.tile([128, 2, NBLK], BF16, tag="hA")
                hB = hbp.tile([128, 2, NBLK], BF16, tag="hB")

                # ---- layer 1 wave 1: h chunks 0-3 for both sets ----
                for c in range(4):
                    nc.tensor.matmul(
                        pH[32 * c:32 * c + 32, 0, 0:NBLK],
                        lhsT=w1_sb[0:49, c], rhs=xA,
                        start=True, stop=True, tile_position=(0, 32 * c))
                    nc.tensor.matmul(
                        pH[32 * c:32 * c + 32, 1, 0:NBLK],
                        lhsT=w1_sb[64:113, c], rhs=xB,
                        start=True, stop=True, tile_position=(64, 32 * c))
                # free the wave-1 banks first, one per engine, in parallel
                nc.vector.tensor_scalar(
                    hA[:, 0], pH[:, 0, 0:NBLK], 0.0, None,
                    mybir.AluOpType.max)
                nc.scalar.activation(
                    out=hB[:, 0], in_=pH[:, 1, 0:NBLK],
                    func=mybir.ActivationFunctionType.Relu)

                # group-16 tails into pV rows 32*(p%4): A in bank 0
                # (row-group-0 tiles), B in bank 1 (row-group-1).
                j4 = 32 * (p % 4)
                nc.tensor.matmul(
                    pV[j4:j4 + 32, 0, 0:NBLK], lhsT=w1_sb[0:49, 8], rhs=xA,
                    start=True, stop=True, tile_position=(0, j4))
                nc.tensor.matmul(
                    pV[j4:j4 + 32, 1, 0:NBLK], lhsT=w1_sb[64:113, 8], rhs=xB,
                    start=True, stop=True, tile_position=(64, j4))

                # ---- layer 1 wave 2: h chunks 4-7 ----
                for c in range(4):
                    nc.tensor.matmul(
                        pH[32 * c:32 * c + 32, 2, 0:NBLK],
                        lhsT=w1_sb[0:49, c + 4], rhs=xA,
                        start=True, stop=True, tile_position=(0, 32 * c))
                    nc.tensor.matmul(
                        pH[32 * c:32 * c + 32, 3, 0:NBLK],
                        lhsT=w1_sb[64:113, c + 4], rhs=xB,
                        start=True, stop=True, tile_position=(64, 32 * c))
                nc.vector.tensor_scalar(
                    hA[:, 1], pH[:, 2, 0:NBLK], 0.0, None,
                    mybir.AluOpType.max)
                nc.scalar.activation(
                    out=hB[:, 1], in_=pH[:, 3, 0:NBLK],
                    func=mybir.ActivationFunctionType.Relu)

                # tail evac + export once per 4 blocks
                if p % 4 == 3:
                    hT = tsb.tile([128, 2, NBLK], BF16, tag="hT")
                    if (p // 4) % 2 == 0:
                        nc.vector.tensor_scalar(
                            hT, pV[:, :, 0:NBLK], 0.0, None,
                            mybir.AluOpType.max)
                    else:
                        nc.scalar.activation(
                            out=hT, in_=pV[:, :, 0:NBLK],
                            func=mybir.ActivationFunctionType.Relu)
                    t4 = p // 4
                    nc.gpsimd.dma_start(
                        o3[:, :, t4 * NBLK:(t4 + 1) * NBLK], hT)

                # ---- layer 2 of the previous block ----
                if prev is not None:
                    pprev = prev[0]
                    if pprev % LBATCH == 0:
                        ob8a = osa.tile([128, LBATCH, NBLK], BF16, tag="ob8a")
                        ob8b = osb.tile([128, LBATCH, NBLK], BF16, tag="ob8b")
                    layer2(pprev, prev[1], prev[2], ob8a, ob8b)
                    if pprev % LBATCH == LBATCH - 1:
                        c8 = slice((pprev - 7) * NBLK, (pprev + 1) * NBLK)
                        for g in range(4):
                            nc.sync.dma_start(o12[g, :, 0, c8],
                                              ob8a[32 * g:32 * g + 12])
                            nc.sync.dma_start(o12[g, :, 1, c8],
                                              ob8b[32 * g:32 * g + 12])
                prev = (p, hA, hB)
              # drain the last block's layer 2
              pprev = prev[0]
              if pprev % LBATCH == 0:
                  ob8a = osa.tile([128, LBATCH, NBLK], BF16, tag="ob8a")
                  ob8b = osb.tile([128, LBATCH, NBLK], BF16, tag="ob8b")
              layer2(pprev, prev[1], prev[2], ob8a, ob8b)
              c8 = slice((pprev - (pprev % LBATCH)) * NBLK,
                         (pprev + 1) * NBLK)
              nb8 = (pprev % LBATCH) + 1
              for g in range(4):
                  nc.sync.dma_start(o12[g, :, 0, c8],
                                    ob8a[32 * g:32 * g + 12, 0:nb8])
                  nc.sync.dma_start(o12[g, :, 1, c8],
                                    ob8b[32 * g:32 * g + 12, 0:nb8])
              prev = None
    nc.finalize()
    return nc


_NC_CACHE = None


def _get_nc():
    global _NC_CACHE
    if _NC_CACHE is None:
        _NC_CACHE = _build_nc()
    return _NC_CACHE


def _kernel_impl(x, W1, b1, W2, b2, idx, _want_trace=False):
    x = np.asarray(x, np.float32)
    w1l, w2l = _host_weights(W1, b1, W2, b2, idx)

    in_maps = []
    for c in range(NCORES):
        xc = x[c * BC:(c + 1) * BC].reshape(TC, NF)
        xt2 = np.empty((KX, S), BF16_NP)
        xt2[0:NF] = np.ascontiguousarray(xc[:S].T)
        xt2[NF] = np.float32(1.0)
        xt2[NF + 1:2 * NF + 1] = np.ascontiguousarray(xc[S:].T)
        xt2[2 * NF + 1] = np.float32(1.0)
        in_maps.append({"x2": xt2, "w1": w1l, "w2": w2l})

    nc = _get_nc()
    res = run_bass_kernel_spmd(
        nc, in_maps, core_ids=list(range(NCORES)), trace=_want_trace,
    )

    b2 = np.asarray(b2, np.float32).reshape(NG * C)
    W2_16 = np.asarray(W2, np.float32)[16]                  # [16, 3]
    out = np.empty((B, T, NG, C), np.float32)
    for c in range(NCORES):
        o12 = np.asarray(res.results[c]["o12"], np.float32)  # [4, 12, 2, S]
        o3 = np.asarray(res.results[c]["o3"], np.float32)    # [128, 2, S4]
        oc = np.empty((TC, NG * C), np.float32)
        # o12[g, :, 0]: {A g0-3, A g8-11, B g0-3, B g8-11}[g]
        # o12[g, :, 1]: {A g4-7, A g12-15, B g4-7, B g12-15}[g]
        oc[:S, 0:12] = o12[0, :, 0].T
        oc[:S, 24:36] = o12[1, :, 0].T
        oc[S:, 0:12] = o12[2, :, 0].T
        oc[S:, 24:36] = o12[3, :, 0].T
        oc[:S, 12:24] = o12[0, :, 1].T
        oc[:S, 36:48] = o12[1, :, 1].T
        oc[S:, 12:24] = o12[2, :, 1].T
        oc[S:, 36:48] = o12[3, :, 1].T
        # o3[32j+k, a, 486t:486t+486] = tail h row k (k<16), set a, block 4t+j
        h4 = o3.reshape(4, 32, 2, S4 // NBLK, NBLK)[:, 0:16]
        g4 = np.einsum("jkats,kc->jatsc", h4, W2_16)  # [j, A/B, t, 486, 3]
        gAB = np.empty((2, S, 3), np.float32)
        for j in range(4):
            for t in range(S4 // NBLK):
                gb = 4 * t + j
                gAB[:, gb * NBLK:(gb + 1) * NBLK] = g4[j, :, t]
        oc[:S, 48:51] = gAB[0]
        oc[S:, 48:51] = gAB[1]
        oc += b2[None, :]
        out[c * BC:(c + 1) * BC] = oc.reshape(BC, T, NG, C)
    return out, res


def kernel(**inputs):
    out, _ = _kernel_impl(**inputs)
    return out


# revision 26
# speedup vs baseline: 1.4342x; 1.4342x over previous
"""Trainium2 Bass kernel for nn_BoneRefusion (17-group BoneMLP over [B,T,16,3]).

v7 design (pure data parallel over batch, 8 cores):
  - Host packs per-core inputs feature-major with 2-set packing:
      x2 [98, S] bf16, S = tokens_per_core/2. Rows 0-47 = set A features,
      row 48 = ones (bakes b1 into layer 1), rows 49-96 = set B,
      row 97 = ones. Column j holds tokens (j, S+j).
  - On chip x lives as [128, N]: set A at partitions 0-48, set B at
    64-112, so the PE runs in 64x32 array-tiling mode: row group 0 = A,
    row group 1 = B, 8 concurrent 64x32 tiles. Blocks are N=486
    pair-columns, 64 blocks per core.
  - A prologue of back-to-back dummy matmuls warms the PE HAM clock
    gate once (4/8 -> 8/8); steady-state gaps are too short to
    re-throttle, so every real matmul runs at 2.4 GHz.
  - PSUM (f32), all 8 banks:
      pAB1 [128,2,512]: A h0-127 | B h0-127
      pAB2 [128,2,512]: A h128-255 | B h128-255
      pV [128,2,512]: group-16 tails of 4 blocks (A | B), rows 32*(p%4)
      pO [128,2,512]: layer-2 outs row-group-0 | row-group-1
  - Evacuation (TRN2 floor: fp32 PSUM = 1 elem/cycle/lane): DVE handles
    the A banks -> hA, ACT the B banks -> hB (separate destination
    tiles per engine avoid cross-engine false dependencies); outs split
    ob8a (ACT) / ob8b (DVE); h and outs leave as bf16.
  - Layer 2 is software-pipelined one block behind layer 1.
  - The group-16 16x3 epilogue runs on the host from the exported tail
    h (o3, gpsimd SWDGE); b2 is added on the host during unshard.
  - Input loads and batched main-out stores ride the sync DGE queue.
"""

import sys

import numpy as np
import ml_dtypes

sys.path.insert(0, "/opt/trn_rl_repo")

import concourse.bass as bass
import concourse.mybir as mybir
import concourse.tile as tile
from concourse import bacc
from concourse.bass_utils import run_bass_kernel_spmd

BF16 = mybir.dt.bfloat16
F32 = mybir.dt.float32
BF16_NP = ml_dtypes.bfloat16

NG = 17          # groups
HID = 16         # hidden per group
B, T, NJ, C = 2048, 243, 16, 3
NF = NJ * C      # 48 input features per token
NCORES = 8
BC = B // NCORES           # batches per core
TC = BC * T                # tokens per core
S = TC // 2                # token pairs per core (2-set packing)
KX = 2 * (NF + 1)          # 98 input rows in DRAM
NBLK = 486                 # pair-columns per block
NBLOCKS = S // NBLK        # 64
LBATCH = 8                 # blocks per input-load / output-store batch
S4 = S // 4                # o3 columns (tail batches of 4 blocks)


def _host_weights(W1, b1, W2, b2, idx):
    """Stationaries: w1l [128, 9*32] bf16, w2l [128, 48] bf16."""
    W1 = np.asarray(W1, np.float32)
    b1 = np.asarray(b1, np.float32)
    W2 = np.asarray(W2, np.float32)
    idx = np.asarray(idx)

    # Scatter per-group [12, 16] W1 blocks into the 48-feature space.
    # Padded limb rows of W1 are already zero, so += handles duplicates.
    w1full = np.zeros((NF, NG * HID), np.float32)
    for g in range(NG):
        for j in range(4):
            r = int(idx[g, j]) * C
            w1full[r:r + C, g * HID:(g + 1) * HID] += W1[g, j * C:(j + 1) * C, :]
    b1flat = b1.reshape(NG * HID)

    # Layer-1 chunk stationaries [49, 32] x 9 (chunk 8 = group-16 tail,
    # cols 16-31 stay zero), duplicated to rows 64-112 for the
    # row-group-1 (set B) array tiles.
    w1l = np.zeros((128, 9 * 32), np.float32)
    for c in range(8):
        w1l[0:NF, 32 * c:32 * c + 32] = w1full[:, 32 * c:32 * c + 32]
        w1l[NF, 32 * c:32 * c + 32] = b1flat[32 * c:32 * c + 32]
    w1l[0:NF, 8 * 32:8 * 32 + 16] = w1full[:, 256:272]
    w1l[NF, 8 * 32:8 * 32 + 16] = b1flat[256:272]
    w1l[64:128, :] = w1l[0:64, :]

    # Layer-2 stationaries: M1-M4 [64, 12] at cols 12k (groups 4k..4k+3),
    # duplicated to rows 64-127.
    w2l = np.zeros((128, 48), np.float32)
    for k in range(4):
        for j in range(4):
            g = 4 * k + j
            w2l[16 * j:16 * j + 16, 12 * k + 3 * j:12 * k + 3 * j + 3] = W2[g]
    w2l[64:128, :] = w2l[0:64, :]

    return w1l.astype(BF16_NP), w2l.astype(BF16_NP)


def _build_nc(repeat=1):
    nc = bacc.Bacc(
        "TRN2", target_bir_lowering=False, debug=False, num_devices=NCORES,
    )
    x2 = nc.dram_tensor("x2", [KX, S], BF16, kind="ExternalInput").ap()
    w1 = nc.dram_tensor("w1", [128, 9 * 32], BF16, kind="ExternalInput").ap()
    w2 = nc.dram_tensor("w2", [128, 48], BF16, kind="ExternalInput").ap()
    # Outputs bf16. o12[g, r, q]: layer-2 outs from psum col group g
    # (partitions 32g..32g+12), q = psum bank (0 = row-group-0 matmuls,
    # 1 = row-group-1). o3[32j+k, a, 486t:486t+486] = relu'd group-16
    # hidden row k (k<16) of set a for block 4t+j.
    o12 = nc.dram_tensor("o12", [4, 12, 2, S], BF16, kind="ExternalOutput").ap()
    o3 = nc.dram_tensor("o3", [128, 2, S4], BF16, kind="ExternalOutput").ap()

    with tile.TileContext(nc) as tc:
        with (
            tc.tile_pool(name="singles", bufs=1) as singles,
            tc.tile_pool(name="xin", bufs=2) as xin,
            tc.tile_pool(name="ha", bufs=2) as hap,
            tc.tile_pool(name="hb", bufs=2) as hbp,
            tc.tile_pool(name="tsb", bufs=2) as tsb,
            tc.tile_pool(name="osa", bufs=2) as osa,
            tc.tile_pool(name="osb", bufs=2) as osb,
            tc.tile_pool(name="hps", bufs=1, space="PSUM") as hps,
            tc.tile_pool(name="vps", bufs=1, space="PSUM") as vps,
        ):
            w1_sb = singles.tile([128, 9, 32], BF16)
            nc.sync.dma_start(w1_sb, w1.rearrange("k (c m) -> k c m", c=9))
            w2_sb = singles.tile([128, 48], BF16)
            nc.sync.dma_start(w2_sb, w2)

            WARMUP = False
            if WARMUP:
                # HAM warm-up: back-to-back matmuls on array tile (0,0)
                # into rotating pH banks (no deps -> pure PE-FIFO stream).
                pH0 = hps.tile([128, 2, 512], F32, tag="pAB1")
                wrm = w1_sb[0:49].rearrange("p c m -> p (c m)")
                for k in range(10):
                    nc.tensor.matmul(
                        pH0[0:32, k % 2, 0:288], lhsT=w1_sb[0:49, 0],
                        rhs=wrm[:, 0:288], start=True, stop=True,
                        tile_position=(0, 0))

            def layer2(p, hA, hB, ob8a, ob8b):
                pO = hps.tile([128, 2, 512], F32, tag="pO")
                nc.tensor.matmul(pO[0:12, 0, 0:NBLK], lhsT=w2_sb[0:64, 0:12],
                                 rhs=hA[0:64, 0], start=True, stop=True,
                                 tile_position=(0, 0))
                nc.tensor.matmul(pO[0:12, 1, 0:NBLK], lhsT=w2_sb[64:128, 12:24],
                                 rhs=hA[64:128, 0], start=True, stop=True,
                                 tile_position=(64, 0))
                nc.tensor.matmul(pO[32:44, 0, 0:NBLK], lhsT=w2_sb[0:64, 24:36],
                                 rhs=hA[0:64, 1], start=True, stop=True,
                                 tile_position=(0, 32))
                nc.tensor.matmul(pO[32:44, 1, 0:NBLK], lhsT=w2_sb[64:128, 36:48],
                                 rhs=hA[64:128, 1], start=True, stop=True,
                                 tile_position=(64, 32))
                nc.tensor.matmul(pO[64:76, 0, 0:NBLK], lhsT=w2_sb[0:64, 0:12],
                                 rhs=hB[0:64, 0], start=True, stop=True,
                                 tile_position=(0, 64))
                nc.tensor.matmul(pO[64:76, 1, 0:NBLK], lhsT=w2_sb[64:128, 12:24],
                                 rhs=hB[64:128, 0], start=True, stop=True,
                                 tile_position=(64, 64))
                nc.tensor.matmul(pO[96:108, 0, 0:NBLK], lhsT=w2_sb[0:64, 24:36],
                                 rhs=hB[0:64, 1], start=True, stop=True,
                                 tile_position=(0, 96))
                nc.tensor.matmul(pO[96:108, 1, 0:NBLK], lhsT=w2_sb[64:128, 36:48],
                                 rhs=hB[64:128, 1], start=True, stop=True,
                                 tile_position=(64, 96))
                # out evac split across engines into per-engine tiles
                nc.scalar.activation(
                    out=ob8a[:, p % LBATCH, :], in_=pO[:, 0, 0:NBLK],
                    func=mybir.ActivationFunctionType.Identity)
                nc.vector.tensor_scalar(
                    ob8b[:, p % LBATCH, :], pO[:, 1, 0:NBLK], 0.0, None,
                    mybir.AluOpType.add)

            prev = None          # (p, hA, hB) awaiting layer 2
            ob8a = ob8b = None
            pV = None
            for _rep in range(repeat):
              for p in range(NBLOCKS):
                if p % LBATCH == 0:
                    xt8 = xin.tile([128, LBATCH * NBLK], BF16, tag="xt")
                    lo = p * NBLK
                    nc.sync.dma_start(xt8[0:49, :],
                                      x2[0:49, lo:lo + LBATCH * NBLK])
                    nc.sync.dma_start(xt8[64:113, :],
                                      x2[49:98, lo:lo + LBATCH * NBLK])
                if p % 4 == 0:
                    pV = vps.tile([128, 2, 512], F32, tag="pV")
                half = p % LBATCH
                xA = xt8[0:49, half * NBLK:(half + 1) * NBLK]
                xB = xt8[64:113, half * NBLK:(half + 1) * NBLK]
                pAB1 = hps.tile([128, 2, 512], F32, tag="pAB1")
                pAB2 = hps.tile([128, 2, 512], F32, tag="pAB2")
                hA = hap.tile([128, 2, NBLK], BF16, tag="hA")
                hB = hbp.tile([128, 2, NBLK], BF16, tag="hB")

                # ---- layer 1 wave 1: h chunks 0-3 for both sets ----
                for c in range(4):
                    nc.tensor.matmul(
                        pAB1[32 * c:32 * c + 32, 0, 0:NBLK],
                        lhsT=w1_sb[0:49, c], rhs=xA,
                        start=True, stop=True, tile_position=(0, 32 * c))
                    nc.tensor.matmul(
                        pAB1[32 * c:32 * c + 32, 1, 0:NBLK],
                        lhsT=w1_sb[64:113, c], rhs=xB,
                        start=True, stop=True, tile_position=(64, 32 * c))
                # free the wave-1 banks first, one per engine, in parallel
                nc.vector.tensor_scalar(
                    hA[:, 0], pAB1[:, 0, 0:NBLK], 0.0, None,
                    mybir.AluOpType.max)
                nc.scalar.activation(
                    out=hB[:, 0], in_=pAB1[:, 1, 0:NBLK],
                    func=mybir.ActivationFunctionType.Relu)

                # group-16 tails into pV rows 32*(p%4): A in bank 0
                # (row-group-0 tiles), B in bank 1 (row-group-1).
                j4 = 32 * (p % 4)
                nc.tensor.matmul(
                    pV[j4:j4 + 32, 0, 0:NBLK], lhsT=w1_sb[0:49, 8], rhs=xA,
                    start=True, stop=True, tile_position=(0, j4))
                nc.tensor.matmul(
                    pV[j4:j4 + 32, 1, 0:NBLK], lhsT=w1_sb[64:113, 8], rhs=xB,
                    start=True, stop=True, tile_position=(64, j4))

                # ---- layer 1 wave 2: h chunks 4-7 ----
                for c in range(4):
                    nc.tensor.matmul(
                        pAB2[32 * c:32 * c + 32, 0, 0:NBLK],
                        lhsT=w1_sb[0:49, c + 4], rhs=xA,
                        start=True, stop=True, tile_position=(0, 32 * c))
                    nc.tensor.matmul(
                        pAB2[32 * c:32 * c + 32, 1, 0:NBLK],
                        lhsT=w1_sb[64:113, c + 4], rhs=xB,
                        start=True, stop=True, tile_position=(64, 32 * c))
                nc.vector.tensor_scalar(
                    hA[:, 1], pAB2[:, 0, 0:NBLK], 0.0, None,
                    mybir.AluOpType.max)
                nc.scalar.activation(
                    out=hB[:, 1], in_=pAB2[:, 1, 0:NBLK],
                    func=mybir.ActivationFunctionType.Relu)

                # tail evac + export once per 4 blocks
                if p % 4 == 3:
                    hT = tsb.tile([128, 2, NBLK], BF16, tag="hT")
                    if (p // 4) % 2 == 0:
                        nc.vector.tensor_scalar(
                            hT, pV[:, :, 0:NBLK], 0.0, None,
                            mybir.AluOpType.max)
                    else:
                        nc.scalar.activation(
                            out=hT, in_=pV[:, :, 0:NBLK],
                            func=mybir.ActivationFunctionType.Relu)
                    t4 = p // 4
                    nc.gpsimd.dma_start(
                        o3[:, :, t4 * NBLK:(t4 + 1) * NBLK], hT)

                # ---- layer 2 of the previous block ----
                if prev is not None:
                    pprev = prev[0]
                    if pprev % LBATCH == 0:
                        ob8a = osa.tile([128, LBATCH, NBLK], BF16, tag="ob8a")
                        ob8b = osb.tile([128, LBATCH, NBLK], BF16, tag="ob8b")
                    layer2(pprev, prev[1], prev[2], ob8a, ob8b)
                    if pprev % LBATCH == LBATCH - 1:
                        c8 = slice((pprev - 7) * NBLK, (pprev + 1) * NBLK)
                        for g in range(4):
                            nc.sync.dma_start(o12[g, :, 0, c8],
                                              ob8a[32 * g:32 * g + 12])
                            nc.sync.dma_start(o12[g, :, 1, c8],
                                              ob8b[32 * g:32 * g + 12])
                prev = (p, hA, hB)
              # drain the last block's layer 2
              pprev = prev[0]
              if pprev % LBATCH == 0:
                  ob8a = osa.tile([128, LBATCH, NBLK], BF16, tag="ob8a")
                  ob8b = osb.tile([128, LBATCH, NBLK], BF16, tag="ob8b")
              layer2(pprev, prev[1], prev[2], ob8a, ob8b)
              c8 = slice((pprev - (pprev % LBATCH)) * NBLK,
                         (pprev + 1) * NBLK)
              nb8 = (pprev % LBATCH) + 1
              for g in range(4):
                  nc.sync.dma_start(o12[g, :, 0, c8],
                                    ob8a[32 * g:32 * g + 12, 0:nb8])
                  nc.sync.dma_start(o12[g, :, 1, c8],
                                    ob8b[32 * g:32 * g + 12, 0:nb8])
              prev = None
    nc.finalize()
    return nc


_NC_CACHE = None


def _get_nc():
    global _NC_CACHE
    if _NC_CACHE is None:
        _NC_CACHE = _build_nc()
    return _NC_CACHE


def _kernel_impl(x, W1, b1, W2, b2, idx, _want_trace=False):
    x = np.asarray(x, np.float32)
    w1l, w2l = _host_weights(W1, b1, W2, b2, idx)

    in_maps = []
    for c in range(NCORES):
        xc = x[c * BC:(c + 1) * BC].reshape(TC, NF)
        xt2 = np.empty((KX, S), BF16_NP)
        xt2[0:NF] = np.ascontiguousarray(xc[:S].T)
        xt2[NF] = np.float32(1.0)
        xt2[NF + 1:2 * NF + 1] = np.ascontiguousarray(xc[S:].T)
        xt2[2 * NF + 1] = np.float32(1.0)
        in_maps.append({"x2": xt2, "w1": w1l, "w2": w2l})

    nc = _get_nc()
    res = run_bass_kernel_spmd(
        nc, in_maps, core_ids=list(range(NCORES)), trace=_want_trace,
    )

    b2 = np.asarray(b2, np.float32).reshape(NG * C)
    W2_16 = np.asarray(W2, np.float32)[16]                  # [16, 3]
    out = np.empty((B, T, NG, C), np.float32)
    for c in range(NCORES):
        o12 = np.asarray(res.results[c]["o12"], np.float32)  # [4, 12, 2, S]
        o3 = np.asarray(res.results[c]["o3"], np.float32)    # [128, 2, S4]
        oc = np.empty((TC, NG * C), np.float32)
        # o12[g, :, 0]: {A g0-3, A g8-11, B g0-3, B g8-11}[g]
        # o12[g, :, 1]: {A g4-7, A g12-15, B g4-7, B g12-15}[g]
        oc[:S, 0:12] = o12[0, :, 0].T
        oc[:S, 24:36] = o12[1, :, 0].T
        oc[S:, 0:12] = o12[2, :, 0].T
        oc[S:, 24:36] = o12[3, :, 0].T
        oc[:S, 12:24] = o12[0, :, 1].T
        oc[:S, 36:48] = o12[1, :, 1].T
        oc[S:, 12:24] = o12[2, :, 1].T
        oc[S:, 36:48] = o12[3, :, 1].T
        # o3[32j+k, a, 486t:486t+486] = tail h row k (k<16), set a, block 4t+j
        h4 = o3.reshape(4, 32, 2, S4 // NBLK, NBLK)[:, 0:16]
        g4 = np.einsum("jkats,kc->jatsc", h4, W2_16)  # [j, A/B, t, 486, 3]
        gAB = np.empty((2, S, 3), np.float32)
        for j in range(4):
            for t in range(S4 // NBLK):
                gb = 4 * t + j
                gAB[:, gb * NBLK:(gb + 1) * NBLK] = g4[j, :, t]
        oc[:S, 48:51] = gAB[0]
        oc[S:, 48:51] = gAB[1]
        oc += b2[None, :]
        out[c * BC:(c + 1) * BC] = oc.reshape(BC, T, NG, C)
    return out, res


def kernel(**inputs):
    out, _ = _kernel_impl(**inputs)
    return out
